# revision 2
# baseline (speedup 1.0000x reference)
"""BinaryLinear 2:4 kernel for trn2 (8 NeuronCores).

Computes: out = binarize(weight * mask_2_4(weight)) @ x
  - mask_2_4: keep 2 largest-|.| of every 4 along the reduction dim
  - binarize: kept positive -> 1.0, else 0.0
  - out = wb @ x, (4096, 4096) fp32

Sharding: 4 (weight rows) x 2 (x cols) grid. Core c=(mi,ni) loads weight
rows [mi*1024, (mi+1)*1024) and x cols [ni*2048, (ni+1)*2048), computes a
(1024, 2048) output block; host assembles the 4x2 blocks. This halves the
per-core HBM traffic vs 8-way row sharding (56MiB vs 80MiB), putting DMA
(~164us) under the tensor-engine floor (~220us).

Per-core pipeline:
  Phase A (mask): for each (128-row, 2048-col) weight chunk compute
    t = 2nd-largest(|w|) per group of 4 via a 7-op min/max tree, then
    wb_j = (w_j >= t) in f16 {0,1}. Since t > 0, the sign condition is
    folded into the >= (exact fp32 compares; verified bit-identical to
    jax top_k mask on this data). DMA-transpose each chunk into k-major
    lhsT layout.
  Phase B (matmul): stream x in 512-col chunks, cast fp32->f16 on the
    scalar engine, accumulate f16 matmuls over K=4096 into one PSUM bank
    (N=512), copy to an SBUF accumulator and store 1MiB blocks.
"""

import numpy as np

# Full problem shapes (hardcoded per contract).
M_FULL = 4096  # outfeatures
K_FULL = 4096  # infeatures (reduction; 2:4 groups along this dim)
N_FULL = 4096  # ncols of x
N_CORES = 8
GRID_M = 4
GRID_N = 2
M_SHARD = M_FULL // GRID_M   # 1024 weight rows per core
N_SHARD = N_FULL // GRID_N   # 2048 x cols per core

_CACHE = {}


def _build_bass(M=M_SHARD, K=K_FULL, N=N_SHARD, k_sub=2048, n_chunk=512,
                xp=2, ogrp=4):
    import concourse.bass as bass
    import concourse.tile as tile
    from concourse import bacc, mybir
    from contextlib import ExitStack

    dt = mybir.dt
    f32 = dt.float32
    f16 = dt.float16
    i32 = dt.int32
    Alu = mybir.AluOpType
    Act = mybir.ActivationFunctionType

    P = 128
    MB = M // P            # 8 m-blocks per core
    KS = K // k_sub        # k chunks for phase A
    KT = K // P            # 32 contraction tiles for matmul
    NCH = N // n_chunk     # 4 x column chunks
    Q = k_sub // 4         # group count within a k chunk
    TPS = k_sub // P       # transposes-worth of k-tiles per chunk

    nc = bacc.Bacc()
    w_d = nc.declare_dram_parameter("w", [M, K], f32, isOutput=False)
    x_d = nc.declare_dram_parameter("x", [K, N], f32, isOutput=False)
    out_d = nc.declare_dram_parameter("out", [M, N], f32, isOutput=True)

    with tile.TileContext(nc) as tc:
        with ExitStack() as ctx:
            wbt_pool = ctx.enter_context(tc.tile_pool(name="wbt", bufs=1))
            paw = ctx.enter_context(tc.tile_pool(name="paw", bufs=2))
            paq = ctx.enter_context(tc.tile_pool(name="paq", bufs=1))
            pawb = ctx.enter_context(tc.tile_pool(name="pawb", bufs=2))
            xs_f = ctx.enter_context(tc.tile_pool(name="xsf", bufs=2))
            xs = ctx.enter_context(tc.tile_pool(name="xs", bufs=2))
            ps_pool = ctx.enter_context(tc.tile_pool(name="ps", bufs=2, space="PSUM"))
            op_pool = ctx.enter_context(tc.tile_pool(name="op", bufs=2))

            # lhsT layout: [k_in(128 partitions), k_tile, m]
            wbt = wbt_pool.tile([P, KT, M], f16)

            # ---------------- Phase A: 2:4 mask + binarize ----------------
            # mb outer so each m-block's lhsT completes early and phase B
            # overlaps the rest of phase A.
            for mb in range(MB):
                for ks in range(KS):
                    wsub = paw.tile([P, k_sub], f32, tag="wsub")
                    nc.gpsimd.dma_start(
                        wsub[:], w_d[mb * P:(mb + 1) * P, ks * k_sub:(ks + 1) * k_sub]
                    )

                    # Exact |w| = clear the sign bit (int32 bitcast AND).
                    aq = paw.tile([P, k_sub], f32, tag="absq")
                    nc.vector.tensor_scalar(
                        aq[:].bitcast(i32), wsub[:].bitcast(i32),
                        0x7FFFFFFF, None, Alu.bitwise_and,
                    )

                    a4 = aq.rearrange("p (g j) -> p g j", j=4)  # (P, Q, 4)
                    w4 = wsub.rearrange("p (g j) -> p g j", j=4)

                    # t = 2nd largest of each |w| group of 4 (exact fp32):
                    #   hi/lo of each pair, then max(min(hi1,hi2), max(lo1,lo2))
                    def tt(name, u, v, op):
                        t_ = paq.tile([P, Q], f32, tag=name)
                        nc.vector.tensor_tensor(t_[:], u, v, op)
                        return t_

                    hi1 = tt("hi1", a4[:, :, 0], a4[:, :, 1], Alu.max)
                    lo1 = tt("lo1", a4[:, :, 0], a4[:, :, 1], Alu.min)
                    hi2 = tt("hi2", a4[:, :, 2], a4[:, :, 3], Alu.max)
                    lo2 = tt("lo2", a4[:, :, 2], a4[:, :, 3], Alu.min)
                    mm = tt("mm", hi1[:], hi2[:], Alu.min)
                    MM = tt("MM", lo1[:], lo2[:], Alu.max)
                    thr = tt("thr", mm[:], MM[:], Alu.max)

                    # wb_j = (w_j >= t) -> f16 {0,1}; t>0 so the kept-positive
                    # condition is folded into the signed compare.
                    wb = pawb.tile([P, k_sub], f16, tag="wbq")
                    wb4 = wb.rearrange("p (g j) -> p g j", j=4)
                    for j in range(4):
                        nc.vector.tensor_tensor(
                            wb4[:, :, j], w4[:, :, j], thr[:], Alu.is_ge
                        )

                    # SBUF->SBUF xbar transpose into lhsT layout
                    # (out[kp, c, m] = wb[m, c*128 + kp]).
                    nc.sync.dma_start_transpose(
                        wbt[:, ks * TPS:(ks + 1) * TPS, mb * P:(mb + 1) * P],
                        wb[:],
                    )

            # ---------------- Phase B: streamed matmul ----------------
            x_r = x_d.rearrange("(ko p) n -> p ko n", p=P)      # (P, KT, N)
            out_r = out_d.rearrange("(mb p) n -> p mb n", p=P)  # (P, MB, N)
            for nch in range(NCH):
                nsl = slice(nch * n_chunk, (nch + 1) * n_chunk)
                xh = xs.tile([P, KT, n_chunk], f16, tag="xh")
                for pc in range(KT // xp):
                    xf = xs_f.tile([P, xp, n_chunk], f32, tag="xf")
                    nc.sync.dma_start(
                        xf[:], x_r[:, pc * xp:(pc + 1) * xp, nsl]
                    )
                    nc.scalar.activation(
                        xh[:, pc * xp:(pc + 1) * xp, :], xf[:], Act.Copy
                    )

                for mbg in range(MB // ogrp):
                    oacc = op_pool.tile([P, ogrp, n_chunk], f32, tag="oacc")
                    for mi in range(ogrp):
                        mb = mbg * ogrp + mi
                        ps = ps_pool.tile([P, n_chunk], f32, tag="ps")
                        for kt in range(KT):
                            nc.tensor.matmul(
                                ps[:],
                                lhsT=wbt[:, kt, mb * P:(mb + 1) * P],
                                rhs=xh[:, kt, :],
                                start=(kt == 0),
                                stop=(kt == KT - 1),
                            )
                        nc.scalar.activation(oacc[:, mi, :], ps[:], Act.Copy)
                    nc.gpsimd.dma_start(
                        out_r[:, mbg * ogrp:(mbg + 1) * ogrp, nsl], oacc[:]
                    )

    nc.finalize()
    return nc


def _get_nc():
    if "nc" not in _CACHE:
        _CACHE["nc"] = _build_bass()
    return _CACHE["nc"]


def make_in_maps(x: np.ndarray, weight: np.ndarray):
    maps = []
    for c in range(N_CORES):
        mi, ni = divmod(c, GRID_N)
        maps.append({
            "w": np.ascontiguousarray(weight[mi * M_SHARD:(mi + 1) * M_SHARD, :]),
            "x": np.ascontiguousarray(x[:, ni * N_SHARD:(ni + 1) * N_SHARD]),
        })
    return maps


def assemble(results) -> np.ndarray:
    out = np.empty((M_FULL, N_FULL), dtype=np.float32)
    for c in range(N_CORES):
        mi, ni = divmod(c, GRID_N)
        out[mi * M_SHARD:(mi + 1) * M_SHARD,
            ni * N_SHARD:(ni + 1) * N_SHARD] = results[c]["out"]
    return out


def kernel(x: np.ndarray, weight: np.ndarray) -> np.ndarray:
    from concourse.bass_utils import run_bass_kernel_spmd

    x = np.ascontiguousarray(np.asarray(x, dtype=np.float32))
    weight = np.ascontiguousarray(np.asarray(weight, dtype=np.float32))
    assert x.shape == (K_FULL, N_FULL) and weight.shape == (M_FULL, K_FULL)

    nc = _get_nc()
    res = run_bass_kernel_spmd(nc, make_in_maps(x, weight), list(range(N_CORES)))
    return assemble(res.results)
